# revision 1
# baseline (speedup 1.0000x reference)
"""GAT-style attention kernel for Trainium2, 8-core row-parallel.

Reference computation:
    h = x @ W; s1 = h @ a1; s2 = h @ a2
    e[i,j] = leaky_relu(s1[j] + s2[i], 0.2); masked by adj; row-softmax; @ h

Algebra: exp(leaky_relu(e)) = max(exp(e), exp(0.2 e)), and any per-row(i)
rescale cancels in the softmax normalization. Dividing by exp(0.2*s2[i])
and factoring exp(s1[j]) out of both max() operands:
    w~[j,i] = exp(s1[j]) * adj[i,j] * max(exp(0.8*s2[i]), exp(-0.8*s1[j]))
    out[i,:] = (sum_j w~[j,i] h[j,:]) / (sum_j w~[j,i])
exp(0.8*s2[i]) is a constant broadcast tile (E2), exp(-0.8*s1[j]) is a
per-partition scalar, and the exp(s1[j]) factor folds into the matmul
weights: h' = exp(s1)*h for the numerator, exp(s1) replacing ones for the
denominator row-sum. The per-chunk score work is then a SINGLE all-bf16 DVE
scalar_tensor_tensor -- no per-chunk ACT exp pass at all.

The mask ships from the host already transposed and widened to bf16
(adjT[j,i] = adj[i,j] as 0.0/1.0): dtype-casting DMAs are SWDGE-only and
run ~500x slower than plain HWDGE transfers, and an on-device transpose
would add 1152 PE ops; bf16 also halves the HBM mask traffic vs int32.
x/xm ship as bf16 too (h is computed in bf16 regardless), halving the
feature traffic.

Per-core fused pipeline over j-chunk groups (DJ=512 j's, i in blocks of
512): the x->h sweep for a group's 4 j-blocks (DMA x on the scalar HWDGE
ring, DVE s1 accum, PE transpose, ACT psum copyback, PE h matmul, ACT exp
of the s1 group, DVE h' scale) runs `skew` groups AHEAD of the mask DMA
(sync ring), the DVE mask op
    wT2 = (E2 max em08[jc]) * adjT          [128j, ROWS i] bf16
and the accumulating PE matmuls (trailing wT2 by one chunk)
    out2[f,i] += h'[jc] @ wT2 ; den[1,i] += esf[jc] @ wT2.
Finalize per i-block: reciprocal of den, PE transpose, scale, store.
build_program(reps=R) replicates the whole pipeline R times in one NEFF
(benchmarking only; the default R=1 is the graded program).

Walrus codegen rejects instructions carrying more than one sync-wait
("Too many sync wait commands"), so after Tile scheduling we legalize the
program: excess waits are moved onto injected same-engine nop instructions
placed immediately before the over-constrained instruction.
"""

import copy
import sys
from contextlib import ExitStack

import numpy as np

if "/opt/trn_rl_repo" not in sys.path:
    sys.path.insert(0, "/opt/trn_rl_repo")

import concourse.bass as bass
import concourse.tile as tile
from concourse import mybir
from concourse.masks import make_identity

P = 128
N_CORES = 8

F32 = mybir.dt.float32
BF16 = mybir.dt.bfloat16
I32 = mybir.dt.int32
AX = mybir.AluOpType
AF = mybir.ActivationFunctionType

# Instruction types whose queue handles multi-wait natively (or that the
# framework emits and walrus already accepts).
_WAIT_SPLIT_SKIP = {"InstHalt", "InstSemWait", "InstEventSemOp"}


def _legalize_waits(nc, template_nop):
    """Move excess sync-waits onto injected same-engine nops."""
    uid = 0
    for f in nc.m.functions:
        for b in f.blocks:
            new_list = []
            changed = False
            for inst in b.instructions:
                si = inst.sync_info
                if (si is not None and len(si.on_wait) > 1
                        and type(inst).__name__ not in _WAIT_SPLIT_SKIP):
                    waits = list(si.on_wait)
                    for w in waits[:-1]:
                        uid += 1
                        nop = copy.copy(template_nop)
                        nop.name = f"I-lwsplit-{uid}"
                        nop.engine = inst.engine
                        nop.sync_info = mybir.SyncInfo(
                            on_wait=[w], on_update=[])
                        try:
                            nop.set_dependency_edges([])
                        except Exception:
                            pass
                        new_list.append(nop)
                    inst.sync_info = mybir.SyncInfo(
                        on_wait=[waits[-1]], on_update=list(si.on_update))
                    changed = True
                new_list.append(inst)
            if changed:
                b.instructions = new_list


def build_program(N=12288, IN_F=256, OUT_F=128, alpha=0.2, legalize=True,
                  dj=512, wt_bufs=5, adj_bufs=3, xb_bufs=12, pack_rsum=True,
                  skew=2, reps=1):
    """Single-core SPMD program. Per-core inputs: adjT [N,ROWS] bf16
    (this core's adj rows, transposed, as 0.0/1.0), x [N,IN_F] f32 (full),
    xm [ROWS,IN_F] f32 (own rows), wx [IN_F,OUT_F] f32 (W), wa1/wa2
    [1,IN_F] f32 (W@a1 / W@a2 rows). Output [ROWS,OUT_F].
    """
    ROWS = N // N_CORES
    NB = N // P
    KB = IN_F // P
    RB = ROWS // P
    IBS = 512 if ROWS % 512 == 0 else P
    IB = ROWS // IBS
    SUBS = IBS // P
    DJ = dj if N % dj == 0 else P
    JCC = N // DJ
    JPC = DJ // P
    assert JPC % 2 == 0, "fused x-sweep pairs j-blocks"

    nc = bass.Bass(trn_type="TRN2")
    adjT_d = nc.dram_tensor("adjT", [N, ROWS], BF16, kind="ExternalInput")
    x_d = nc.dram_tensor("x", [N, IN_F], BF16, kind="ExternalInput")
    xm_d = nc.dram_tensor("xm", [ROWS, IN_F], BF16, kind="ExternalInput")
    wx_d = nc.dram_tensor("wx", [IN_F, OUT_F], F32, kind="ExternalInput")
    wa1_d = nc.dram_tensor("wa1", [1, IN_F], F32, kind="ExternalInput")
    wa2_d = nc.dram_tensor("wa2", [1, IN_F], F32, kind="ExternalInput")
    out_d = nc.dram_tensor("out", [ROWS, OUT_F], F32, kind="ExternalOutput")

    adjT_v = adjT_d[:, :].rearrange("(g p) i -> p g i", p=P)

    with tile.TileContext(nc) as tc:
        template_nop = nc.sync.nop(nofuse=True).ins
        for rep in range(reps):
            _emit_pipeline(nc, tc, rep, N, IN_F, OUT_F, alpha, dj, wt_bufs,
                           adj_bufs, xb_bufs, pack_rsum, skew,
                           adjT_d, x_d, xm_d, wx_d, wa1_d, wa2_d, out_d)

    if legalize:
        _legalize_waits(nc, template_nop)
    return nc


def _emit_pipeline(nc, tc, rep, N, IN_F, OUT_F, alpha, dj, wt_bufs, adj_bufs,
                   xb_bufs, pack_rsum, skew,
                   adjT_d, x_d, xm_d, wx_d, wa1_d, wa2_d, out_d):
    ROWS = N // N_CORES
    NB = N // P
    KB = IN_F // P
    RB = ROWS // P
    IBS = 512 if ROWS % 512 == 0 else P
    IB = ROWS // IBS
    SUBS = IBS // P
    DJ = dj if N % dj == 0 else P
    JCC = N // DJ
    JPC = DJ // P
    R = f"r{rep}_"

    adjT_v = adjT_d[:, :].rearrange("(g p) i -> p g i", p=P)
    with ExitStack() as ctx:
        const = ctx.enter_context(tc.tile_pool(name=R + "const", bufs=1))
        ident_f = const.tile([P, P], F32)
        make_identity(nc, ident_f[:])
        ones1_f = const.tile([1, P], F32)
        nc.gpsimd.memset(ones1_f[:], 1.0)
        ident_b = const.tile([P, P], BF16)
        make_identity(nc, ident_b[:])

        h_sb = const.tile([P, NB * OUT_F], BF16)        # h' = exp(s1)*h
        s1_sb = const.tile([P, NB], F32)
        esf_sb = const.tile([P, NB], F32)               # exp(s1[j])
        esfb_sb = const.tile([P, NB], BF16)             # exp(s1[j]) bf16
        em8_sb = const.tile([P, NB], F32)               # exp(-0.8*s1[j])
        E2 = const.tile([P, ROWS], BF16)                # exp(0.8*s2[i])
        wxb = const.tile([P, KB * OUT_F], BF16)
        wa2b = const.tile([P, IN_F], F32)
        wa1b = const.tile([P, IN_F], F32)

        # ---------------- pre phase: wa1/wa2 bcast, s2, E2 ----------------
        with tc.tile_pool(name=R + "pre_ps", bufs=2, space="PSUM") as pre_ps, \
             tc.tile_pool(name=R + "pre_sb", bufs=3) as pre_sb:
            wxf = pre_sb.tile([P, KB, OUT_F], F32, tag="wxf")
            nc.sync.dma_start(
                wxf[:], wx_d[:, :].rearrange("(c p) f -> p c f", p=P))
            nc.vector.tensor_copy(wxb[:], wxf[:].rearrange("p c f -> p (c f)"))

            nbc = [0]

            def bcast_row(dst, src_row, width):
                # dst[p, :width] = src_row[0, :width] for all 128 partitions
                for q in range(0, width, 512):
                    w = min(512, width - q)
                    ps = pre_ps.tile([P, 512], F32, tag="bc",
                                     name=f"bc_{nbc[0]}")
                    nbc[0] += 1
                    nc.tensor.matmul(ps[:, :w], ones1_f[:],
                                     src_row[0:1, q:q + w],
                                     start=True, stop=True)
                    nc.scalar.copy(dst[:, q:q + w], ps[:, :w])

            wa2_sb = pre_sb.tile([1, IN_F], F32, tag="wa2")
            nc.sync.dma_start(wa2_sb[:], wa2_d[:, :])
            bcast_row(wa2b, wa2_sb, IN_F)
            wa1_sb = pre_sb.tile([1, IN_F], F32, tag="wa1")
            nc.sync.dma_start(wa1_sb[:], wa1_d[:, :])
            bcast_row(wa1b, wa1_sb, IN_F)

            # s2 of this core's rows (exact f32 on DVE); one batched load
            s2_loc = pre_sb.tile([P, RB], F32, tag="s2loc")
            xm_t = pre_sb.tile([P, RB, IN_F], BF16, tag="xm")
            nc.scalar.dma_start(
                xm_t[:], xm_d[:, :].rearrange("(s p) f -> p s f", p=P))
            for rb in range(RB):
                junk = pre_sb.tile([P, IN_F], F32, tag="junk")
                nc.vector.scalar_tensor_tensor(
                    junk[:], xm_t[:, rb, :], 1.0, wa2b[:],
                    op0=AX.mult, op1=AX.mult,
                    accum_out=s2_loc[:, rb:rb + 1])
            s2T_ps = pre_ps.tile([RB, P], F32, tag="trs")
            nc.tensor.transpose(s2T_ps[:], s2_loc[:], ident_f[:])
            s2T_sb = pre_sb.tile([RB, P], F32, tag="trs_sb")
            nc.vector.tensor_copy(s2T_sb[:], s2T_ps[:])
            s2row = pre_sb.tile([1, ROWS], F32, tag="s2row")
            # scalar ring: the sync ring already carries the multi-MB adj
            # prefetches, which would delay this tiny E2-critical transfer
            nc.scalar.dma_start(s2row[:], s2T_sb[:])
            # E2 = exp(0.8*s2[i]) broadcast to all partitions
            for q in range(0, ROWS, 512):
                w = min(512, ROWS - q)
                ps = pre_ps.tile([P, 512], F32, tag="bc", name=f"e2_{q}")
                nc.tensor.matmul(ps[:, :w], ones1_f[:], s2row[0:1, q:q + w],
                                 start=True, stop=True)
                nc.scalar.activation(E2[:, q:q + w], ps[:, :w], AF.Exp,
                                     scale=1.0 - alpha)

        # ---------------- main pools ----------------
        ps_out = ctx.enter_context(tc.tile_pool(name=R + "ps_out", bufs=1, space="PSUM"))
        ps_rs = ctx.enter_context(tc.tile_pool(name=R + "ps_rs", bufs=1, space="PSUM"))
        adj_pool = ctx.enter_context(tc.tile_pool(name=R + "adj", bufs=adj_bufs))
        wt_pool = ctx.enter_context(tc.tile_pool(name=R + "wt", bufs=wt_bufs))
        xs_sb = ctx.enter_context(tc.tile_pool(name=R + "xs_sb", bufs=xb_bufs))

        out2 = [ps_out.tile([P, IBS], F32, tag=f"o{b}", name=f"out2_{b}")
                for b in range(IB)]
        if pack_rsum:
            rs_all = ps_rs.tile([P, IBS], F32, name="rs_all")
            rsum = [rs_all[32 * b:32 * b + 1, :] for b in range(IB)]
        else:
            rsum = [ps_rs.tile([1, IBS], F32, tag=f"r{b}", name=f"rsum_{b}")[:]
                    for b in range(IB)]

        def emit_matmuls(pjc, pw, pfirst, plast):
            for b in range(IB):
                nc.tensor.matmul(
                    out2[b][:], h_sb[:, pjc * OUT_F:(pjc + 1) * OUT_F],
                    pw[:, b * IBS:(b + 1) * IBS], start=pfirst, stop=plast)
            for b in range(IB):
                nc.tensor.matmul(rsum[b], esfb_sb[:, pjc:pjc + 1],
                                 pw[:, b * IBS:(b + 1) * IBS],
                                 start=pfirst, stop=plast)

        # ---------------- fused main loop ----------------
        with tc.tile_pool(name=R + "xs_ps", bufs=2, space="PSUM") as xs_ps:
            pending = []
            prevs = []
            adj_tiles = {}
            with tc.high_priority():
                for jcc in range(min(2, JCC)):
                    t = adj_pool.tile([P, JPC, ROWS], BF16, tag="adj_bf",
                                      name=f"adjb_{jcc}")
                    nc.sync.dma_start(
                        t[:], adjT_v[:, jcc * JPC:(jcc + 1) * JPC, :])
                    adj_tiles[jcc] = t

            def emit_group(pg0, padj):
                for js in range(JPC):
                    jc = pg0 + js
                    first, last = jc == 0, jc == NB - 1
                    wT = wt_pool.tile([P, ROWS], BF16, tag="wT",
                                      name=f"wT_{jc}")
                    nc.vector.scalar_tensor_tensor(
                        wT[:], E2[:], em8_sb[:, jc:jc + 1], padj[:, js, :],
                        op0=AX.max, op1=AX.mult)
                    pending.append((jc, wT, first, last))
                    if len(pending) > 1:
                        emit_matmuls(*pending.pop(0))

            for jcc in range(JCC):
                if jcc in adj_tiles:
                    adj_bf = adj_tiles.pop(jcc)
                else:
                    adj_bf = adj_pool.tile([P, JPC, ROWS], BF16, tag="adj_bf",
                                           name=f"adjb_{jcc}")
                    nc.sync.dma_start(
                        adj_bf[:], adjT_v[:, jcc * JPC:(jcc + 1) * JPC, :])
                # x sweep, stage 1: load this group's x blocks, s1 accum
                g0 = jcc * JPC
                xbs = []
                for js in range(JPC):
                    jb = g0 + js
                    xb = xs_sb.tile([P, IN_F], BF16, tag="xb",
                                    name=f"xb_{jb}")
                    # scalar (ACT) HWDGE ring: keeps the small
                    # latency-critical x loads off the adj-prefetch ring
                    nc.scalar.dma_start(xb[:], x_d[jb * P:(jb + 1) * P, :])
                    junk2 = xs_sb.tile([P, IN_F], F32, tag="junk2",
                                       name=f"j2_{jb}")
                    nc.vector.scalar_tensor_tensor(
                        junk2[:], xb[:], 1.0, wa1b[:],
                        op0=AX.mult, op1=AX.mult,
                        accum_out=s1_sb[:, jb:jb + 1])
                    xbs.append(xb)
                # group scalars: exp(s1), bf16 copy, exp(-0.8*s1)
                nc.scalar.activation(esf_sb[:, g0:g0 + JPC],
                                     s1_sb[:, g0:g0 + JPC], AF.Exp)
                nc.vector.tensor_copy(esfb_sb[:, g0:g0 + JPC],
                                      esf_sb[:, g0:g0 + JPC])
                nc.scalar.activation(em8_sb[:, g0:g0 + JPC],
                                     s1_sb[:, g0:g0 + JPC], AF.Exp,
                                     scale=-(1.0 - alpha))
                # stage 2: prescale x rows by exp(s1[j]) on ACT (SBUF only),
                # transpose, and matmul -- h lands in h_sb already scaled
                for jp in range(JPC // 2):
                    xT_ps = xs_ps.tile([P, 2, KB, P], BF16, tag="xT",
                                       name=f"xT_{jcc}_{jp}")
                    for u in range(2):
                        js = jp * 2 + u
                        jb = g0 + js
                        xbp = xs_sb.tile([P, IN_F], BF16, tag="xbp",
                                         name=f"xbp_{jb}")
                        nc.scalar.activation(xbp[:], xbs[js][:], AF.Copy,
                                             scale=esf_sb[:, jb:jb + 1])
                        for k2 in range(KB):
                            nc.tensor.transpose(
                                xT_ps[:, u, k2, :],
                                xbp[:, k2 * P:(k2 + 1) * P], ident_b[:])
                    xT_sb = xs_sb.tile([P, 2, KB, P], BF16, tag="xTs",
                                       name=f"xTs_{jcc}_{jp}")
                    nc.scalar.copy(
                        xT_sb[:].rearrange("p u c f -> p (u c f)"),
                        xT_ps[:].rearrange("p u c f -> p (u c f)"))
                    h_ps = xs_ps.tile([P, 2, OUT_F], F32, tag="h",
                                      name=f"h_{jcc}_{jp}")
                    for u in range(2):
                        for k2 in range(KB):
                            nc.tensor.matmul(
                                h_ps[:, u, :], xT_sb[:, u, k2, :],
                                wxb[:, k2 * OUT_F:(k2 + 1) * OUT_F],
                                start=(k2 == 0), stop=(k2 == KB - 1))
                    nc.scalar.copy(
                        h_sb[:, (g0 + jp * 2) * OUT_F:(g0 + jp * 2 + 2) * OUT_F],
                        h_ps[:].rearrange("p u f -> p (u f)"))
                # mask + max + matmuls for a PREVIOUS group: its em8/adj
                # are long ready, so the DVE never waits on this group's
                # x-chain (skew-group software pipeline)
                prevs.append((g0, adj_bf))
                if len(prevs) > skew:
                    emit_group(*prevs.pop(0))
            while prevs:
                emit_group(*prevs.pop(0))
            while pending:
                emit_matmuls(*pending.pop(0))

        # ---------------- finalize ----------------
        with tc.tile_pool(name=R + "fin_ps", bufs=4, space="PSUM") as fin_ps, \
             tc.tile_pool(name=R + "fin", bufs=5) as fin_pool:
            for b in range(IB):
                o_sb = fin_pool.tile([P, IBS], F32, tag="osb")
                nc.scalar.copy(o_sb[:], out2[b][:])
                rs_sb = fin_pool.tile([1, IBS], F32, tag="rssb")
                nc.scalar.copy(rs_sb[:], rsum[b])
                rall = fin_pool.tile([P, SUBS], F32, tag="rall")
                for t in range(SUBS):
                    rT_ps = fin_ps.tile([P, 512], BF16, tag="tr",
                                        name=f"rT_{b}_{t}")
                    rT = rT_ps[:, 0:2].bitcast(F32)
                    nc.tensor.matmul(rT[:, 0:1],
                                     rs_sb[0:1, t * P:(t + 1) * P],
                                     ones1_f[0:1, 0:1], start=True, stop=True)
                    nc.vector.tensor_copy(rall[:, t:t + 1], rT[:, 0:1])
                rinv = fin_pool.tile([P, SUBS], F32, tag="rinv")
                nc.vector.reciprocal(rinv[:], rall[:])
                for t in range(SUBS):
                    oT_ps = fin_ps.tile([P, 512], BF16, tag="tr",
                                        name=f"oT_{b}_{t}")
                    oT = oT_ps[:, 0:256].bitcast(F32)
                    nc.tensor.transpose(oT[:], o_sb[:, t * P:(t + 1) * P],
                                        ident_f[:])
                    fin = fin_pool.tile([P, OUT_F], F32, tag="fint")
                    nc.vector.tensor_scalar_mul(fin[:], oT[:, :OUT_F],
                                                rinv[:, t:t + 1])
                    nc.sync.dma_start(
                        out_d[b * IBS + t * P:b * IBS + (t + 1) * P, :],
                        fin[:])


_PROG_CACHE = {}


def _get_program(N, IN_F, OUT_F):
    key = (N, IN_F, OUT_F)
    if key not in _PROG_CACHE:
        _PROG_CACHE[key] = build_program(N, IN_F, OUT_F)
    return _PROG_CACHE[key]


def make_in_maps(x, adj, W, a1, a2):
    import ml_dtypes

    N, IN_F = x.shape
    ROWS = N // N_CORES
    wx = np.ascontiguousarray(W, dtype=np.float32)
    wa1 = np.ascontiguousarray((W @ a1)[None, :], dtype=np.float32)
    wa2 = np.ascontiguousarray((W @ a2)[None, :], dtype=np.float32)
    # adj as bf16 bit patterns (0/1 are exact), still [i, j] layout.
    adj_u16 = adj.astype(np.uint16) * np.uint16(0x3F80)
    x_bf = np.ascontiguousarray(x.astype(ml_dtypes.bfloat16))
    in_maps = []
    for c in range(N_CORES):
        sl = slice(c * ROWS, (c + 1) * ROWS)
        adjT = np.ascontiguousarray(adj_u16[sl].T).view(ml_dtypes.bfloat16)
        in_maps.append({
            "adjT": adjT,
            "x": x_bf,
            "xm": np.ascontiguousarray(x_bf[sl]),
            "wx": wx,
            "wa1": wa1,
            "wa2": wa2,
        })
    return in_maps


def kernel(x, adj, W, a1, a2, trace=False):
    x = np.asarray(x, dtype=np.float32)
    adj = np.ascontiguousarray(np.asarray(adj, dtype=np.int32))
    W = np.asarray(W, dtype=np.float32)
    a1 = np.asarray(a1, dtype=np.float32)
    a2 = np.asarray(a2, dtype=np.float32)
    N, IN_F = x.shape
    OUT_F = W.shape[1]

    from concourse.bass_utils import run_bass_kernel_spmd

    nc = _get_program(N, IN_F, OUT_F)
    in_maps = make_in_maps(x, adj, W, a1, a2)
    res = run_bass_kernel_spmd(
        nc, in_maps, core_ids=list(range(N_CORES)), trace=trace)
    out = np.concatenate([r["out"] for r in res.results], axis=0)
    kernel.last_results = res
    return out

